# revision 7
# baseline (speedup 1.0000x reference)
"""CPC loss kernel for Trainium2 (8 NeuronCores, SPMD data-parallel over batch).

Device computes the dominant work: per (batch, step) the [T, K] negative-logit
matmul (ce @ negT) and the numerically-shifted exp + sum over K, producing
sum_k exp(neg_logit_k - C) per position.  Host does the index gather for the
negative table (per the sharding hint), the positive-logit dot products
(3% of FLOPs), and the final masked scalar reduction.
"""

import json
import os
import re

import ml_dtypes
import numpy as np

B, T, E, S, K = 64, 512, 256, 5, 128
NCORES = 8
BLOC = B // NCORES
C_SHIFT = 40.0  # exp shift; |logits| <~ 16*6 keeps exp(l - C) finite in f32

_CACHE = {}

# run_bass_kernel_spmd results of the last device run (for test harness use)
last_results = None


def _split_multi_waits(bir_bytes):
    """walrus in this container accepts at most one sync-wait command per
    instruction; split extra waits onto same-engine NoOps inserted before."""
    bj = json.loads(bir_bytes)
    ctr = 0
    for f in bj["functions"]:
        for blk in f["blocks"]:
            insts = blk.get("instructions")
            if not insts:
                continue
            out = []
            for ins in insts:
                si = ins.get("sync_info")
                if si:
                    waits = si.get("on_wait") or []
                    if len(waits) > 1:
                        for w in waits[:-1]:
                            ctr += 1
                            nop = {
                                "engine": ins["engine"],
                                "ins": [],
                                "outs": [],
                                "name": f"I-wsplit-{ctr}",
                                "opcode": "NoOp",
                                "sync_info": {"on_wait": [w]},
                            }
                            if "debug" in ins:
                                nop["debug"] = ins["debug"]
                            out.append(nop)
                        si["on_wait"] = [waits[-1]]
                out.append(ins)
            blk["instructions"] = out
    return json.dumps(bj).encode()


def _build_bass_real():
    import concourse.bass as bass
    import concourse.mybir as mybir
    import concourse.tile as tile
    from concourse.vector_clock import ScopedClock
    import bass_rust as _br

    class _TC(tile.TileContext):
        def _drain_and_barrier(self, tick_clock, wait_clock):
            gc = tick_clock.global_clock
            arr = [int(s) for s in re.findall(r"\d+", repr(gc))]
            emitted = False
            for p, t in enumerate(arr):
                if t > 0:
                    sub = _br.VectorClock(
                        [t if q == p else 0 for q in range(len(arr))]
                    )
                    d = self.nc.sync.drain()
                    wait_clock.add_sem_waits(d.ins, ScopedClock({None: sub}))
                    emitted = True
            if not emitted:
                self.nc.sync.drain()
            self.nc.all_engine_barrier()
            popped = self.nc._tile_sem_poison_stack.pop()
            assert popped is self._sem_poison
            self.nc.clear_and_free_semaphores(list(self.sems.allocated().values()))
            self.nc.all_engine_barrier()

    bf16 = mybir.dt.bfloat16
    f32 = mybir.dt.float32
    Exp = mybir.ActivationFunctionType.Exp
    X = mybir.AxisListType.X

    nc = bass.Bass("TRN2")
    cet = nc.dram_tensor("cet", [BLOC, S, 128, 2, T], bf16, kind="ExternalInput")
    negt = nc.dram_tensor("negt", [128, BLOC, 2, K], bf16, kind="ExternalInput")
    seout = nc.dram_tensor("seout", [128, BLOC, S, 4], f32, kind="ExternalOutput")

    with _TC(nc) as tc:
        with (
            tc.tile_pool(name="negs", bufs=1) as negp,
            tc.tile_pool(name="ce", bufs=4) as cep,
            tc.tile_pool(name="ps", bufs=6, space="PSUM") as pp,
            tc.tile_pool(name="ex", bufs=4) as ep,
            tc.tile_pool(name="acc", bufs=1) as accp,
        ):
            neg_sb = negp.tile([128, BLOC, 2, K], bf16)
            nc.sync.dma_start(out=neg_sb, in_=negt[:, :, :, :])
            bias_sb = negp.tile([128, 1], f32)
            nc.vector.memset(bias_sb, -C_SHIFT)
            out_acc = accp.tile([128, BLOC, S, 4], f32)
            for b in range(BLOC):
                for i in range(S):
                    ce_sb = cep.tile([128, 2, T], bf16, tag="ce")
                    nc.sync.dma_start(out=ce_sb, in_=cet[b, i])
                    ps = pp.tile([128, 4, K], f32, tag="ps")
                    for j in range(4):
                        for h in range(2):
                            nc.tensor.matmul(
                                ps[:, j, :],
                                ce_sb[:, h, j * 128 : (j + 1) * 128],
                                neg_sb[:, b, h, :],
                                start=(h == 0),
                                stop=(h == 1),
                            )
                    ex = ep.tile([128, 4, K], f32, tag="ex")
                    nc.scalar.activation(out=ex, in_=ps, func=Exp, bias=bias_sb[:, :])
                    nc.vector.reduce_sum(out=out_acc[:, b, i, :], in_=ex, axis=X)
            nc.sync.dma_start(out=seout[:, :, :, :], in_=out_acc)

    orig_to_json = nc.to_json_bytes
    nc.to_json_bytes = lambda: _split_multi_waits(orig_to_json())
    return nc


def _get_nc():
    if "nc" not in _CACHE:
        _CACHE["nc"] = _build_bass_real()
    return _CACHE["nc"]


def kernel(base_payload, mapped_ctx_payload, seq_lens, sample_ids):
    global last_results
    from concourse.bass_utils import run_bass_kernel_spmd

    base = np.ascontiguousarray(np.asarray(base_payload, dtype=np.float32))
    mc = np.asarray(mapped_ctx_payload, dtype=np.float32)
    sl = np.asarray(seq_lens).astype(np.int64)
    sid = np.asarray(sample_ids).astype(np.int64)

    # ---- host prep -------------------------------------------------------
    # negative table gather: [B, K, E]
    neg = base.reshape(B * T, E)[sid]
    # negt[c][p, b, h, k] = neg[c*BLOC + b, k, h*128 + p]
    negt = np.ascontiguousarray(
        neg.reshape(B, K, 2, 128).transpose(3, 0, 2, 1)
    ).astype(ml_dtypes.bfloat16)  # [128, B, 2, K]

    # cet[b, i, p, h, t] = mc[b, t, h*128+p, i]
    cet = np.ascontiguousarray(
        mc.transpose(0, 3, 2, 1)  # [b, i, e, t]
        .reshape(B, S, 2, 128, T)
        .transpose(0, 1, 3, 2, 4)
    ).astype(ml_dtypes.bfloat16)  # [B, S, 128, 2, T]

    # positive logits pos[b, i-1, t] for t < T-i (zero elsewhere)
    pos = np.zeros((B, S, T), np.float32)
    for i in range(1, S + 1):
        pos[:, i - 1, : T - i] = np.einsum(
            "bte,bte->bt", mc[:, : T - i, :, i - 1], base[:, i:, :], optimize=True
        )

    # ---- device ----------------------------------------------------------
    nc = _get_nc()
    in_maps = [
        {
            "cet": np.ascontiguousarray(cet[c * BLOC : (c + 1) * BLOC]),
            "negt": np.ascontiguousarray(negt[:, c * BLOC : (c + 1) * BLOC]),
        }
        for c in range(NCORES)
    ]
    res = run_bass_kernel_spmd(nc, in_maps, core_ids=list(range(NCORES)))
    last_results = res
    se = np.stack([r["seout"] for r in res.results])  # [C, 128, BLOC, S, 4]
    # -> [B, S, T] with t = j*128 + p
    se_full = (
        se.transpose(0, 2, 3, 4, 1).reshape(B, S, T).astype(np.float64)
    )

    # ---- host combine ----------------------------------------------------
    lse_neg = np.log(se_full) + C_SHIFT  # [B, S, T]
    lse = np.logaddexp(lse_neg, pos.astype(np.float64))
    d = lse - pos  # per-position loss, garbage at masked tail

    t_idx = np.arange(T)[None, None, :]
    steps = np.arange(1, S + 1)[None, :, None]
    mask = t_idx < (sl[:, None, None] - steps)  # [B, S, T]
    sums = np.where(mask, d, 0.0).sum(axis=(0, 2))  # [S]
    cnts = mask.sum(axis=(0, 2)).astype(np.float64)
    loss = (sums / cnts).mean()
    return np.asarray(loss, dtype=np.float32)


# revision 9
# speedup vs baseline: 1.0646x; 1.0646x over previous
"""CPC loss kernel for Trainium2 (8 NeuronCores, SPMD data-parallel over batch).

Device computes the dominant work: per (batch, step) the [T, K] negative-logit
matmul (ce @ negT) and the numerically-shifted exp + sum over K, producing
sum_k exp(neg_logit_k - C) per position.  Host does the index gather for the
negative table (per the sharding hint), the positive-logit dot products
(3% of FLOPs), and the final masked scalar reduction.
"""

import json
import os
import re

import ml_dtypes
import numpy as np

B, T, E, S, K = 64, 512, 256, 5, 128
NCORES = 8
BLOC = B // NCORES
C_SHIFT = 40.0  # exp shift; |logits| <~ 16*6 keeps exp(l - C) finite in f32

_CACHE = {}

# run_bass_kernel_spmd results of the last device run (for test harness use)
last_results = None


def _split_multi_waits(bir_bytes):
    """walrus in this container accepts at most one sync-wait command per
    instruction; split extra waits onto same-engine NoOps inserted before."""
    bj = json.loads(bir_bytes)
    ctr = 0
    for f in bj["functions"]:
        for blk in f["blocks"]:
            insts = blk.get("instructions")
            if not insts:
                continue
            out = []
            for ins in insts:
                si = ins.get("sync_info")
                if si:
                    waits = si.get("on_wait") or []
                    if len(waits) > 1:
                        for w in waits[:-1]:
                            ctr += 1
                            nop = {
                                "engine": ins["engine"],
                                "ins": [],
                                "outs": [],
                                "name": f"I-wsplit-{ctr}",
                                "opcode": "NoOp",
                                "sync_info": {"on_wait": [w]},
                            }
                            if "debug" in ins:
                                nop["debug"] = ins["debug"]
                            out.append(nop)
                        si["on_wait"] = [waits[-1]]
                out.append(ins)
            blk["instructions"] = out
    return json.dumps(bj).encode()


def _build_bass_real():
    import concourse.bass as bass
    import concourse.mybir as mybir
    import concourse.tile as tile
    from concourse.vector_clock import ScopedClock
    import bass_rust as _br

    class _TC(tile.TileContext):
        def _drain_and_barrier(self, tick_clock, wait_clock):
            gc = tick_clock.global_clock
            arr = [int(s) for s in re.findall(r"\d+", repr(gc))]
            emitted = False
            for p, t in enumerate(arr):
                if t > 0:
                    sub = _br.VectorClock(
                        [t if q == p else 0 for q in range(len(arr))]
                    )
                    d = self.nc.sync.drain()
                    wait_clock.add_sem_waits(d.ins, ScopedClock({None: sub}))
                    emitted = True
            if not emitted:
                self.nc.sync.drain()
            self.nc.all_engine_barrier()
            popped = self.nc._tile_sem_poison_stack.pop()
            assert popped is self._sem_poison
            self.nc.clear_and_free_semaphores(list(self.sems.allocated().values()))
            self.nc.all_engine_barrier()

    bf16 = mybir.dt.bfloat16
    f32 = mybir.dt.float32
    Exp = mybir.ActivationFunctionType.Exp
    X = mybir.AxisListType.X

    nc = bass.Bass("TRN2")
    cet = nc.dram_tensor("cet", [BLOC, 128, S, 2, T], bf16, kind="ExternalInput")
    negt = nc.dram_tensor("negt", [128, BLOC, 2, K], bf16, kind="ExternalInput")
    seout = nc.dram_tensor("seout", [128, BLOC, S, 4], f32, kind="ExternalOutput")

    G = 2  # steps per PSUM tile / exp batch
    batches = [(0, 2), (2, 2), (4, 1)]

    with _TC(nc) as tc:
        with (
            tc.tile_pool(name="negs", bufs=1) as negp,
            tc.tile_pool(name="ce", bufs=3) as cep,
            tc.tile_pool(name="ps", bufs=3, space="PSUM") as pp,
            tc.tile_pool(name="ex", bufs=4) as ep,
            tc.tile_pool(name="acc", bufs=1) as accp,
        ):
            neg_sb = negp.tile([128, BLOC, 2, K], bf16)
            nc.sync.dma_start(out=neg_sb, in_=negt[:, :, :, :])
            bias_sb = negp.tile([128, 1], f32)
            nc.vector.memset(bias_sb, -C_SHIFT)
            out_acc = accp.tile([128, BLOC, S, 4], f32)
            for b in range(BLOC):
                ce_sb = cep.tile([128, S, 2, T], bf16, tag="ce")
                nc.sync.dma_start(out=ce_sb, in_=cet[b])
                for i0, g in batches:
                    ps = pp.tile([128, G, 4, K], f32, tag="ps")
                    for gi in range(g):
                        i = i0 + gi
                        for j in range(4):
                            for h in range(2):
                                nc.tensor.matmul(
                                    ps[:, gi, j, :],
                                    ce_sb[:, i, h, j * 128 : (j + 1) * 128],
                                    neg_sb[:, b, h, :],
                                    start=(h == 0),
                                    stop=(h == 1),
                                )
                    ex = ep.tile([128, G, 4, K], bf16, tag="ex")
                    nc.scalar.activation(
                        out=ex[:, :g], in_=ps[:, :g], func=Exp, bias=bias_sb[:, :]
                    )
                    nc.vector.reduce_sum(
                        out=out_acc[:, b, i0 : i0 + g, :], in_=ex[:, :g], axis=X
                    )
            nc.sync.dma_start(out=seout[:, :, :, :], in_=out_acc)

    orig_to_json = nc.to_json_bytes
    nc.to_json_bytes = lambda: _split_multi_waits(orig_to_json())
    return nc


def _get_nc():
    if "nc" not in _CACHE:
        _CACHE["nc"] = _build_bass_real()
    return _CACHE["nc"]


def kernel(base_payload, mapped_ctx_payload, seq_lens, sample_ids):
    global last_results
    from concourse.bass_utils import run_bass_kernel_spmd

    base = np.ascontiguousarray(np.asarray(base_payload, dtype=np.float32))
    mc = np.asarray(mapped_ctx_payload, dtype=np.float32)
    sl = np.asarray(seq_lens).astype(np.int64)
    sid = np.asarray(sample_ids).astype(np.int64)

    # ---- host prep -------------------------------------------------------
    # negative table gather: [B, K, E]
    neg = base.reshape(B * T, E)[sid]
    # negt[c][p, b, h, k] = neg[c*BLOC + b, k, h*128 + p]
    negt = np.ascontiguousarray(
        neg.reshape(B, K, 2, 128).transpose(3, 0, 2, 1)
    ).astype(ml_dtypes.bfloat16)  # [128, B, 2, K]

    # cet[b, p, i, h, t] = mc[b, t, h*128+p, i]  (10KB contiguous per
    # partition line for full-rate DMA)
    cet = np.ascontiguousarray(
        mc.transpose(0, 2, 3, 1)  # [b, e, i, t]
        .reshape(B, 2, 128, S, T)
        .transpose(0, 2, 3, 1, 4)
    ).astype(ml_dtypes.bfloat16)  # [B, 128, S, 2, T]

    # positive logits pos[b, i-1, t] for t < T-i (zero elsewhere)
    pos = np.zeros((B, S, T), np.float32)
    for i in range(1, S + 1):
        pos[:, i - 1, : T - i] = np.einsum(
            "bte,bte->bt", mc[:, : T - i, :, i - 1], base[:, i:, :], optimize=True
        )

    # ---- device ----------------------------------------------------------
    nc = _get_nc()
    in_maps = [
        {
            "cet": np.ascontiguousarray(cet[c * BLOC : (c + 1) * BLOC]),
            "negt": np.ascontiguousarray(negt[:, c * BLOC : (c + 1) * BLOC]),
        }
        for c in range(NCORES)
    ]
    res = run_bass_kernel_spmd(nc, in_maps, core_ids=list(range(NCORES)))
    last_results = res
    se = np.stack([r["seout"] for r in res.results])  # [C, 128, BLOC, S, 4]
    # -> [B, S, T] with t = j*128 + p
    se_full = (
        se.transpose(0, 2, 3, 4, 1).reshape(B, S, T).astype(np.float64)
    )

    # ---- host combine ----------------------------------------------------
    lse_neg = np.log(se_full) + C_SHIFT  # [B, S, T]
    lse = np.logaddexp(lse_neg, pos.astype(np.float64))
    d = lse - pos  # per-position loss, garbage at masked tail

    t_idx = np.arange(T)[None, None, :]
    steps = np.arange(1, S + 1)[None, :, None]
    mask = t_idx < (sl[:, None, None] - steps)  # [B, S, T]
    sums = np.where(mask, d, 0.0).sum(axis=(0, 2))  # [S]
    cnts = mask.sum(axis=(0, 2)).astype(np.float64)
    loss = (sums / cnts).mean()
    return np.asarray(loss, dtype=np.float32)
